# revision 30
# baseline (speedup 1.0000x reference)
"""Bass/Trainium2 kernel for BayesianDropoutLayer:
    out = X @ (mask[:, None] * M) + m
  X [8192, 2048] f32, M [2048, 2048] f32, m [2048] f32, mask [2048] i32.

Data-parallel over batch across 8 NeuronCores (one [1024, 2048] output
shard per core), ~139 us HW exec (max core; best cores run a gap-free
~110 us matmul stream + fixed overheads) vs the 149 us baseline:

  - host-side contraction pruning: mask zeroes ~10% of M's rows; those
    k-rows contribute exactly 0, so the host gathers only the surviving
    rows (padded to a multiple of 128) of M and X^T. K drops 2048 -> 1920
    (15 k-tiles), cutting PE work and load traffic by 1/16 and removing
    all on-device mask handling. Numerically exact (dropping +0.0 terms).
  - orientation: stationary = M subtile [128k, 128u], moving = X^T chunk
    [128k, 512b]; out tiles are [128u, 512b] (units on partitions), so the
    bias is a per-partition scalar added during PSUM eviction (no PE
    broadcast matmuls). fp32r keeps the PE at 1 column/cycle (~227 ns per
    N=512 matmul incl. weight-swap overhead).
  - 4 unit-phases of 512 units; in phases 0-2 the 8 PSUM-bank chains
    accumulate kt-OUTER (interleaved across banks) so the PE consumes each
    (mw_kt, xt_kt) pair as it streams in; the single sync-queue load
    stream is ordered [mw0_0, xt_0, mw0_1, xt_1, ...] to match (a single
    HWDGE ring sustains ~400 GB/s; splitting queues splits bandwidth).
    The last phase runs kt-inner so chains finish staggered and the final
    evictions/stores hide behind remaining matmuls.
  - every DMA source is a fully contiguous DRAM block: the host lays mw
    out as four per-phase k-major tensors [n_kt, 128, 512] (a single
    [K, 2048] layout made each phase-slice load a strided walk of 2 KiB
    lines at 8 KiB pitch - measurably slower DRAM reads).
  - evictions alternate Vector/Scalar engines (PSUM -> SBUF + bias add)
    so each phase's 8 evictions land before the next phase reuses the
    banks; stores ride the idle GpSimd engine's queue.
  - 14 warmup matmuls on a memset tile run during the DMA head: the load
    stream cannot outrun the PE's phase-0 burn until ~12 us of transfers
    have accumulated, so the junk matmuls bridge exactly that window --
    releasing the HAM clock gate (cold = 1.2 GHz) and letting the real
    stream start warm and gap-free. Oversizing is safe (phase 0 is
    DMA-bound); undersizing costs cold-clock oscillation."""

import sys

if "/opt/trn_rl_repo" not in sys.path:
    sys.path.insert(0, "/opt/trn_rl_repo")

import numpy as np

import concourse.bass as bass  # noqa: F401
import concourse.mybir as mybir
import concourse.tile as tile
from concourse import bacc
from concourse.bass_utils import run_bass_kernel_spmd

P = 128
BATCH = 8192
N_IN = 2048
UNITS = 2048
N_CORES = 8
B_SHARD = BATCH // N_CORES
NPAN = 4
UP = UNITS // NPAN
NB = 512
NUT = UNITS // P

F32 = mybir.dt.float32
F32R = mybir.dt.float32r

_CACHED = {}


def _build_nc(n_kt):
    if n_kt in _CACHED:
        return _CACHED[n_kt]

    k_pad = n_kt * P
    nc = bacc.Bacc("TRN2", target_bir_lowering=False, debug=False)

    xt = nc.dram_tensor("xt", [k_pad, B_SHARD], F32R, kind="ExternalInput")
    # per-phase k-major mw tensors: every load is a fully contiguous DRAM
    # block (the single [k_pad, UNITS] layout made each mw load a strided
    # walk of 2 KiB lines at 8 KiB pitch — poor DRAM page locality)
    mwp_d = [
        nc.dram_tensor(f"mwp{pn}", [n_kt, P, UP], F32R, kind="ExternalInput")
        for pn in range(NPAN)
    ]
    biasd = nc.dram_tensor("biasd", [P, NUT], F32, kind="ExternalInput")
    # out[ut, bc, p, n]: unit ut*128+p, batch-row bc*512+n — each store is a
    # fully contiguous 256 KiB block and the kernel's final store is small
    out = nc.dram_tensor("out", [NUT, 2, P, NB], F32, kind="ExternalOutput")

    xt3 = xt.rearrange("(kt p) b -> p kt b", p=P)

    groups = []
    g0 = 0
    while g0 < n_kt:
        gs = min(4, n_kt - g0)
        groups.append((g0, gs))
        g0 += gs

    with tile.TileContext(nc) as tc:
        with (
            tc.tile_pool(name="xtp", bufs=1) as xtp,
            tc.tile_pool(name="mwp", bufs=1) as mwp,
            tc.tile_pool(name="mwgp", bufs=1) as mwgp,
            tc.tile_pool(name="misc", bufs=1) as misc,
            tc.tile_pool(name="outp", bufs=1) as outp,
            tc.tile_pool(name="psum", bufs=8, space="PSUM") as psump,
        ):
            bias_sb = misc.tile([P, NUT], F32)
            nc.scalar.dma_start(bias_sb[:], biasd[:, :])
            wt = misc.tile([P, NB], F32R)
            nc.vector.memset(wt[:].bitcast(mybir.dt.uint32), 0)

            wps = psump.tile([P, NB], F32, tag="ps", name="wps")
            for _ in range(14):
                nc.tensor.matmul(wps[:], wt[:, 0:P], wt[:, :], start=True, stop=True)

            mw_tiles = {pn: [None] * n_kt for pn in range(NPAN)}
            xts = []
            for kt in range(n_kt):
                m0 = mwp.tile([P, UP], F32R, name=f"mw0_{kt}")
                nc.sync.dma_start(m0[:], mwp_d[0][kt, :, :])
                mw_tiles[0][kt] = m0[:, :]
                x = xtp.tile([P, B_SHARD], F32R, name=f"xt_{kt}")
                nc.sync.dma_start(x[:], xt3[:, kt, :])
                xts.append(x)
            for pn in range(1, NPAN):
                mwp3 = mwp_d[pn].rearrange("kt p n -> p kt n")
                for (gs0, gsz) in groups:
                    t = mwgp.tile(
                        [P, gsz, UP], F32R, tag="mwg", bufs=8,
                        name=f"mwg{pn}_{gs0}",
                    )
                    nc.sync.dma_start(t[:], mwp3[:, gs0 : gs0 + gsz, :])
                    for j in range(gsz):
                        mw_tiles[pn][gs0 + j] = t[:, j, :]

            def evict_one(pn, ul, bc, ps_t):
                # PSUM -> SBUF with bias, then store the 256 KiB chunk right
                # away; engines alternate per chunk so pairs run concurrently
                ut = pn * 4 + ul
                ob = outp.tile(
                    [P, NB], F32, tag="ob", bufs=8, name=f"ob{ut}_{bc}"
                )
                if (2 * ul + bc) % 2 == 0:
                    nc.vector.tensor_scalar_add(
                        ob[:], ps_t[:], bias_sb[:, ut : ut + 1]
                    )
                else:
                    nc.scalar.add(ob[:], ps_t[:], bias_sb[:, ut : ut + 1])
                nc.gpsimd.dma_start(out[ut, bc, :, :], ob[:])

            def evict_store(pn, ul, ps_pair):
                for bc in range(2):
                    evict_one(pn, ul, bc, ps_pair[bc])

            for pn in range(NPAN):
                mwt = mw_tiles[pn]
                ps = [
                    psump.tile([P, NB], F32, tag="ps", name=f"ps{pn}_{i}")
                    for i in range(8)
                ]
                if pn < NPAN - 1:
                    for kt in range(n_kt):
                        st = kt == 0
                        sp = kt == n_kt - 1
                        for ul in range(4):
                            lhsT = mwt[kt][:, ul * P : (ul + 1) * P]
                            nc.tensor.matmul(
                                ps[2 * ul][:], lhsT, xts[kt][:, 0:NB],
                                start=st, stop=sp,
                            )
                            nc.tensor.matmul(
                                ps[2 * ul + 1][:], lhsT, xts[kt][:, NB : 2 * NB],
                                start=st, stop=sp,
                            )
                    for ul in range(4):
                        evict_store(pn, ul, (ps[2 * ul], ps[2 * ul + 1]))
                else:
                    for ul in range(4):
                        for bc in range(2):
                            for kt in range(n_kt):
                                st = kt == 0
                                sp = kt == n_kt - 1
                                lhsT = mwt[kt][:, ul * P : (ul + 1) * P]
                                nc.tensor.matmul(
                                    ps[2 * ul + bc][:], lhsT,
                                    xts[kt][:, bc * NB : (bc + 1) * NB],
                                    start=st, stop=sp,
                                )
                            evict_one(pn, ul, bc, ps[2 * ul + bc])

    nc.compile()
    _CACHED[n_kt] = nc
    return nc


def _prep(X, M, m, mask):
    mask = np.asarray(mask, dtype=np.int32).reshape(N_IN)
    keep = np.flatnonzero(mask != 0)
    n_kt = max(1, -(-len(keep) // P))
    k_pad = n_kt * P
    if len(keep) < k_pad:
        pad = np.flatnonzero(mask == 0)[: k_pad - len(keep)]
        idx = np.concatenate([keep, pad])
    else:
        idx = keep
    mw = np.asarray(M, dtype=np.float32)[idx]
    if len(keep) < k_pad:
        mw[len(keep):] = 0.0
    mwk = mw.reshape(n_kt, P, UNITS)
    mws = tuple(
        np.ascontiguousarray(mwk[:, :, pn * UP : (pn + 1) * UP])
        for pn in range(NPAN)
    )
    bias2d = np.ascontiguousarray(
        np.asarray(m, dtype=np.float32).reshape(NUT, P).T
    )
    return n_kt, idx, mws, bias2d


def run_sharded(X, M, m, mask, trace=False, trace_cores=None):
    n_kt, idx, mws, bias2d = _prep(X, M, m, mask)
    nc = _build_nc(n_kt)
    X = np.asarray(X, dtype=np.float32)
    in_maps = []
    for c in range(N_CORES):
        xs = X[c * B_SHARD : (c + 1) * B_SHARD]
        xtc = np.ascontiguousarray(xs.T[idx])
        im = {"xt": xtc, "biasd": bias2d}
        for pn in range(NPAN):
            im[f"mwp{pn}"] = mws[pn]
        in_maps.append(im)
    res = run_bass_kernel_spmd(
        nc, in_maps, list(range(N_CORES)), trace=trace, trace_cores=trace_cores
    )
    shards = [
        np.transpose(r["out"], (1, 3, 0, 2)).reshape(B_SHARD, UNITS)
        for r in res.results
    ]
    out = np.ascontiguousarray(np.concatenate(shards, axis=0))
    return out, res


def kernel(X, M, m, mask):
    out, _ = run_sharded(X, M, m, mask)
    return out
